# revision 17
# baseline (speedup 1.0000x reference)
"""DCNv4 Trainium2 Bass kernel (v4, software-pipelined).

Data-parallel over batch: sample b runs on core b. Per-sample pipeline:
  1. conv-om via wide-stream implicit GEMM: per 4-row group and (ky, cb),
     one matmul with lhsT = [128c, 96] (3 kx-tap weight blocks at
     32-aligned columns) streaming 4 padded rows (264 cols) -> psum
     [96, 264].  x is DMA'd in 8 pieces so the conv starts early.
  2. merge+transpose+bias in ONE small matmul per 128-pixel tile:
     scalar casts the 3 shifted psum blocks to SBUF oms [97, 256] f16
     (row 96 = ones), then ptall[pix, 32t+o] = oms.T @ S97 where S97
     stacks [I32;I32;I32] + a bias row -> pixel-major offsets staying
     in PSUM (the DVE chain reads them there; no copy-out).  Offset
     channels are host-permuted (x 0..8, y 9..17, mod 18..26) so reads
     are stride-1.  mm1 (yT = (w_out @ x)^T) interleaves on the PE.
  3. bilinear math in f16 RELATIVE coords on DVE (f32-internal +2^23+16
     round trick; floor(x+off) = x + floor(off) since pixel coords are
     integers); exact bin-dedup via separable outer products
     A[p, 9sy, 9sx] = sum_k Ry_k (x) Cx_k, with k=7,8 on GPSIMD.
     Border validity is a CONSTANT mask applied to A in one multiply.
     Runs in 4 chunks of [10,10,8,4] tiles, each emitted right after
     its conv groups; the small last chunk shortens the serial tail.
  4. per 2-tile batch one collision-free GPSIMD local_scatter writes A
     into the skewed band Askew[p, d], d = p_local + 64*sy' + sx' + OFS;
     per chunk ONE XBAR DMA-transpose turns the slabs into sbT[q, s, p].
  5. per out-tile t, slab s: out2[p, c] += contraction(sbT slab,
     yT q-block) on the PE, placed a few groups after its chunk.
  6. scalar copies psum -> ot; 4-tile batched DMA to DRAM pixel-major
     [4096, 256]; host transposes and adds b_out.
"""

import sys

import numpy as np

for _p in ("/opt/trn_rl_repo",):
    if _p not in sys.path:
        sys.path.insert(0, _p)

import concourse.bass as bass
import concourse.mybir as mybir
from concourse import bacc
import concourse.tile as tile
from concourse import bass_utils

F32 = mybir.dt.float32
F16 = mybir.dt.float16
I16 = mybir.dt.int16

H = W = 64
HW = H * W
C = 256
NT = 32          # pixel tiles of 128 (2 image rows each)
NK = 9           # sample points
NB = 7           # bins per axis (shifts -3..3)
NBB = NB * NB
NPAD = 50        # per-tile A slots (49 bins + 1 pad)
OFS = 256        # skew offset; q = 128*t + d - OFS
D = 640          # skew width (5 slabs of 128)
NSLAB = 5
CHB = [0, 6, 14, 22, 28, 32]   # chunk tile boundaries (even)

# channel permutation: x-offsets, y-offsets, modulators contiguous
PERM = list(range(0, 18, 2)) + list(range(1, 18, 2)) + list(range(18, 27))


def _make_consts():
    iota2d = np.tile((np.arange(NB, dtype=np.float16) - 3.0)[:, None], (1, 2 * NK))
    pl = np.arange(128)
    sy, sx = np.meshgrid(np.arange(NB), np.arange(NB), indexing="ij")
    srel = (64 * (sy - 3) + (sx - 3) + OFS).reshape(-1)           # [49]
    scidx = np.full((128, 2, NPAD), -1, np.int16)
    for j in range(2):
        scidx[:, j, :NBB] = (pl[:, None] + srel[None, :] + j * D).astype(np.int16)
    # constant validity mask: pixel = 128*t + p -> y = 2t + (p>=64),
    # x = p % 64; corner at grid (sy, sx) has coords (y+sy-3, x+sx-3)
    t_ = np.arange(NT)
    yy = 2 * t_[None, :] + (pl[:, None] // 64)                    # [128, NT]
    xx = (pl % 64)[:, None] + np.zeros((1, NT), np.int64)         # [128, NT]
    cy = yy[:, :, None, None] + (sy - 3)[None, None]              # [128,NT,7,7]
    cx_ = xx[:, :, None, None] + (sx - 3)[None, None]
    mask = ((cy >= 0) & (cy < H) & (cx_ >= 0) & (cx_ < W)).astype(np.float16)
    return {
        "iota2d": np.ascontiguousarray(np.tile(iota2d.reshape(1, NB * 2 * NK), (128, 1))),
        "scidx": np.ascontiguousarray(scidx.reshape(128, 2 * NPAD)),
        "maskc": np.ascontiguousarray(mask.reshape(128, NT * NBB)),
    }


def _make_weights(w_off, b_off, w_mod, b_mod, w_out, b_out):
    wom = np.concatenate([np.asarray(w_off), np.asarray(w_mod)], 0)  # [27,256,3,3]
    wom = wom[PERM]
    bom = np.concatenate([np.asarray(b_off), np.asarray(b_mod)], 0)[PERM]
    # womt96[c, cb, ky, kx, o] = wom[o, cb*128+c, ky, kx]; each kx block
    # padded 27->32 columns so psum blocks land on 32-aligned partitions.
    w81 = np.transpose(wom.reshape(27, 2, 128, 3, 3), (2, 1, 3, 4, 0))
    w96 = np.zeros((128, 2, 3, 3, 32), np.float32)
    w96[:, :, :, :, :27] = w81
    # S97: [97, 32]; rows 32kx+o pick block kx col o; row 96 adds bias
    s97 = np.zeros((97, 32), np.float16)
    for kx in range(3):
        s97[32 * kx:32 * kx + 32, :] = np.eye(32, dtype=np.float16)
    s97[96, :27] = bom.astype(np.float16)
    woutt = np.asarray(w_out).reshape(C, C).T.copy()          # [cin, cout]
    return {
        "womt96": np.ascontiguousarray(w96.reshape(128, 2 * 3 * 96), np.float16),
        "s97": np.ascontiguousarray(s97),
        "woutt": np.ascontiguousarray(woutt, np.float16),
    }


def _build(nc: bass.Bass):
    AOp = mybir.AluOpType
    AF = mybir.ActivationFunctionType

    x_d = nc.dram_tensor("x", [C, HW], F16, kind="ExternalInput").ap()
    womt96_d = nc.dram_tensor("womt96", [128, 2 * 3 * 96], F16, kind="ExternalInput").ap()
    s97_d = nc.dram_tensor("s97", [97, 32], F16, kind="ExternalInput").ap()
    woutt_d = nc.dram_tensor("woutt", [C, C], F16, kind="ExternalInput").ap()
    iota_d = nc.dram_tensor("iota2d", [128, NB * 2 * NK], F16, kind="ExternalInput").ap()
    scidx_d = nc.dram_tensor("scidx", [128, 2 * NPAD], I16, kind="ExternalInput").ap()
    maskc_d = nc.dram_tensor("maskc", [128, NT * NBB], F16, kind="ExternalInput").ap()
    out_d = nc.dram_tensor("out", [HW, C], F16, kind="ExternalOutput").ap()

    with tile.TileContext(nc) as tc:
        with (
            tc.tile_pool(name="per", bufs=1) as per,
            tc.tile_pool(name="ps", bufs=1, space="PSUM") as psp,
            tc.tile_pool(name="rot", bufs=3) as rot,
            tc.tile_pool(name="outp", bufs=3) as outp,
        ):
            # persistent SBUF tensors
            xpad = [per.tile([128, 66 * 66], F16, tag=f"xpad{i}", name=f"xpad{i}") for i in range(2)]
            womt96 = per.tile([128, 2 * 3 * 96], F16, tag="womt96", name="womt96")
            s97 = per.tile([97, 32], F16, tag="s97", name="s97")
            woutt = per.tile([128, 2 * C], F16, tag="woutt", name="woutt")
            iota2 = per.tile([128, NB * 2 * NK], F16, tag="iota2", name="iota2")
            scidx = per.tile([128, 2 * NPAD], I16, tag="scidx", name="scidx")
            maskc = per.tile([128, NT * NBB], F16, tag="maskc", name="maskc")
            oms = per.tile([97, 2 * 256], F16, tag="oms", name="oms")
            yh = per.tile([128, NT * C], F16, tag="yh", name="yh")
            askew = per.tile([128, NT * D], F16, tag="askew", name="askew")
            xh = [per.tile([128, HW], F16, tag=f"xh{i}", name=f"xh{i}") for i in range(2)]
            ahd = per.tile([128, NT * NPAD], F16, tag="ahd", name="ahd")
            ahg = per.tile([128, NT * NBB], F16, tag="ahg", name="ahg")
            tta = per.tile([128, NT * NBB], F16, tag="tta", name="tta")
            ttg = per.tile([128, NT * NBB], F16, tag="ttg", name="ttg")
            # pixel-major conv outputs stay in PSUM ([p, 32t+o], f32)
            ptall = psp.tile([128, NT * 32], F32, tag="ptall", name="ptall")

            K2 = 2 * NK
            names = ("r2", "t02", "bxy", "fxy", "g2")
            b = {n: per.tile([128, NT * K2], F16, tag=f"b_{n}", name=f"b_{n}") for n in names}
            eq2 = per.tile([128, NT * NB * K2], F16, tag="eq2", name="eq2")
            t1c = per.tile([128, NT * (NB - 1) * K2], F16, tag="t1c", name="t1c")
            cxry = per.tile([128, NT * NB * K2], F16, tag="cxry", name="cxry")

            # conv weights first (small, needed immediately); x in 8
            # pieces per cb so xpad rows 0-8 and conv group 0 start after
            # ~1/8 of the transfer; xh doubles as mm1 lhsT.
            nc.sync.dma_start(out=womt96[:], in_=womt96_d)
            nc.sync.dma_start(out=s97[:], in_=s97_d)
            xsrc = x_d.rearrange("(cb p) q -> cb p q", p=128)
            for ch in range(8):
                q0, q1 = 512 * ch, 512 * (ch + 1)
                nc.sync.dma_start(out=xh[0][:, q0:q1], in_=xsrc[0][:, q0:q1])
                nc.scalar.dma_start(out=xh[1][:, q0:q1], in_=xsrc[1][:, q0:q1])
                if ch == 2:
                    nc.sync.dma_start(
                        out=woutt[:].rearrange("p (t o) -> p t o", o=C),
                        in_=woutt_d.rearrange("(t p) o -> p t o", p=128))
            for cb in range(2):
                x3 = xpad[cb][:].rearrange("p (y x) -> p y x", x=66)
                nc.vector.memset(x3[:, 0, :], 0.0)
                nc.vector.memset(x3[:, 65, :], 0.0)
                nc.vector.memset(x3[:, 1:65, 0], 0.0)
                nc.vector.memset(x3[:, 1:65, 65], 0.0)
            for ch in range(8):
                r0, r1 = 8 * ch, 8 * (ch + 1)
                for cb in range(2):
                    x3 = xpad[cb][:].rearrange("p (y x) -> p y x", x=66)
                    src = xh[cb][:].rearrange("p (y x) -> p y x", x=64)
                    nc.vector.tensor_copy(x3[:, 1 + r0:1 + r1, 1:65],
                                          src[:, r0:r1, :])

            # remaining constants (sync; needed only by the chunk pipeline)
            nc.sync.dma_start(out=iota2[:], in_=iota_d)
            nc.sync.dma_start(out=scidx[:], in_=scidx_d)
            nc.sync.dma_start(out=maskc[:], in_=maskc_d)

            nc.vector.memset(oms[96:97, :], 1.0)

            TT = nc.vector.tensor_tensor
            TS = nc.vector.tensor_scalar
            STT = nc.vector.scalar_tensor_tensor
            GTT = nc.gpsimd.tensor_tensor

            # pad slot (49) is read (and discarded) by the scatter
            nc.vector.memset(
                ahd[:].rearrange("p (t s) -> p t s", s=NPAD)[:, :, NBB], 0.0)

            omt3 = ptall[:].rearrange("p (t o) -> p t o", o=32)

            def conv_group(g):
                pom = psp.tile([96, 264], F32, tag="pom", name="pom", bufs=2)
                first = True
                for ky in range(3):
                    for cb in range(2):
                        lhsT = womt96[:, (cb * 3 + ky) * 96:(cb * 3 + ky + 1) * 96]
                        r0 = (4 * g + ky) * 66
                        rhs = xpad[cb][:, r0:r0 + 264]
                        nc.tensor.matmul(pom[:], lhsT, rhs, start=first,
                                         stop=(ky == 2 and cb == 1))
                        first = False
                base = (g % 2) * 256
                for kx in range(3):
                    pv = pom[32 * kx:32 * kx + 32, :] \
                        .rearrange("p (r c) -> p r c", c=66)[:, :, kx:kx + 64]
                    dst = oms[32 * kx:32 * kx + 32, base:base + 256] \
                        .rearrange("p (r c) -> p r c", c=64)
                    nc.scalar.activation(dst, pv, AF.Copy)
                for h2 in range(2):
                    t = 2 * g + h2
                    nc.tensor.matmul(ptall[:, t * 32:(t + 1) * 32],
                                     oms[:, base + h2 * 128:base + h2 * 128 + 128],
                                     s97[:], start=True, stop=True)

            def mm1(t):
                # processes the tile PAIR (t, t+1): one [128, 512] psum tile
                # and ONE scalar copy-out for both
                py = psp.tile([128, 2 * C], F32, tag="py", name="py", bufs=2)
                for h in range(2):
                    for cb in range(2):
                        lhsT = xh[cb][:, (t + h) * 128:(t + h + 1) * 128]
                        nc.tensor.matmul(py[:, h * C:(h + 1) * C], lhsT,
                                         woutt[:, cb * C:(cb + 1) * C],
                                         start=(cb == 0), stop=(cb == 1))
                nc.scalar.activation(yh[:, t * C:(t + 2) * C], py[:], AF.Copy)

            sbT = {}

            def chunk_front(chk):
                """bilinear chain + eq + outer + scatter + transpose."""
                ta, tb = CHB[chk], CHB[chk + 1]
                tn = tb - ta
                tsl = slice(ta, tb)
                ksl = slice(ta * NK, tb * NK)
                bsl = slice(ta * NB * NK, tb * NB * NK)
                b1sl = slice(ta * (NB - 1) * NK, tb * (NB - 1) * NK)
                ssl = slice(ta * NBB, tb * NBB)

                om3c = omt3[:, tsl]
                oxy = om3c[:, :, 0:18]
                mmod = om3c[:, :, 18:27]
                K2 = 2 * NK
                ksl2 = slice(ta * K2, tb * K2)
                bsl2 = slice(ta * NB * K2, tb * NB * K2)
                b1sl2 = slice(ta * (NB - 1) * K2, tb * (NB - 1) * K2)
                fl = lambda ap_: ap_[:, ksl2]
                yv2 = lambda ap_: ap_[:, ksl2] \
                    .rearrange("p (t kk) -> p t kk", kk=K2)[:, :, NK:K2]

                # floor/frac for BOTH axes in one set of ops.  DVE computes
                # in fp32 internally: +2^23+16 forces rounding of s (in
                # (-4,4)) to an integer.
                RC = float(2 ** 23) + 16.0
                TS(fl(b["r2"][:]), oxy, RC, RC, AOp.add, AOp.subtract)
                STT(fl(b["t02"][:]), oxy, 0.0, fl(b["r2"][:]),
                    AOp.add, AOp.is_lt)
                TT(fl(b["bxy"][:]), fl(b["r2"][:]), fl(b["t02"][:]), AOp.subtract)
                TT(fl(b["fxy"][:]), oxy, fl(b["bxy"][:]), AOp.subtract)
                TS(fl(b["g2"][:]), fl(b["fxy"][:]), -1.0, 1.0, AOp.mult, AOp.add)
                # fold modulation into the y halves in place
                TT(yv2(b["g2"][:]), yv2(b["g2"][:]), mmod, AOp.mult)
                TT(yv2(b["fxy"][:]), yv2(b["fxy"][:]), mmod, AOp.mult)

                # eq + R/C for both axes at once: [128, t, bin, kk];
                # kk 0..8 -> Cx (x bins), kk 9..17 -> Ry (y bins, mod-folded)
                bkv = lambda ap_: ap_[:, bsl2] \
                    .rearrange("p (t b kk) -> p t b kk", b=NB, kk=K2)
                kv_b = lambda ap_: ap_[:, ksl2] \
                    .rearrange("p (t kk) -> p t kk", kk=K2) \
                    .unsqueeze(2).broadcast_to((128, tn, NB, K2))
                io_b = iota2[:].rearrange("q (b kk) -> q b kk", kk=K2) \
                    .unsqueeze(1).broadcast_to((128, tn, NB, K2))

                TT(bkv(eq2[:]), kv_b(b["bxy"][:]), io_b, AOp.is_equal)
                TT(bkv(cxry[:]), bkv(eq2[:]), kv_b(b["g2"][:]), AOp.mult)
                tv = t1c[:, b1sl2].rearrange("p (t b kk) -> p t b kk",
                                             b=NB - 1, kk=K2)
                TT(tv, bkv(eq2[:])[:, :, :NB - 1],
                   kv_b(b["fxy"][:])[:, :, :NB - 1], AOp.mult)
                TT(bkv(cxry[:])[:, :, 1:], bkv(cxry[:])[:, :, 1:], tv, AOp.add)

                # outer products: A[p, t, sy, sx] = sum_k ry_k (x) cx_k
                a_v = ahd[:].rearrange("p (t s) -> p t s", s=NPAD) \
                    [:, tsl, :NBB] \
                    .rearrange("p t (sy sx) -> p t sy sx", sy=NB, sx=NB)
                ag_v = ahg[:, ssl].rearrange("p (t sy sx) -> p t sy sx",
                                             sy=NB, sx=NB)
                ta_v = tta[:, ssl].rearrange("p (t sy sx) -> p t sy sx",
                                             sy=NB, sx=NB)
                tg_v = ttg[:, ssl].rearrange("p (t sy sx) -> p t sy sx",
                                             sy=NB, sx=NB)
                m_v = maskc[:, ssl].rearrange("p (t sy sx) -> p t sy sx",
                                              sy=NB, sx=NB)

                def ocx(k):
                    return bkv(cxry[:])[:, :, :, k].unsqueeze(2) \
                        .broadcast_to((128, tn, NB, NB))

                def ory(k):
                    return bkv(cxry[:])[:, :, :, NK + k].unsqueeze(3) \
                        .broadcast_to((128, tn, NB, NB))

                GTT(ag_v, ory(7), ocx(7), AOp.mult)
                GTT(tg_v, ory(8), ocx(8), AOp.mult)
                GTT(ag_v, ag_v, tg_v, AOp.add)
                GTT(tg_v, ory(6), ocx(6), AOp.mult)
                for k in range(6):
                    if k == 0:
                        TT(a_v, ory(0), ocx(0), AOp.mult)
                    else:
                        TT(ta_v, ory(k), ocx(k), AOp.mult)
                        TT(a_v, a_v, ta_v, AOp.add)
                # k=6 partial from GPSIMD, then k=7,8 partial + border mask
                TT(a_v, a_v, tg_v, AOp.add)
                TT(a_v, a_v, ag_v, AOp.add)
                TT(a_v, a_v, m_v, AOp.mult)

                # skewed scatters, then ONE batched XBAR transpose
                for bt in range(ta // 2, tb // 2):
                    nc.gpsimd.local_scatter(
                        askew[:, bt * 2 * D:(bt + 1) * 2 * D],
                        ahd[:, bt * 2 * NPAD:(bt + 1) * 2 * NPAD],
                        scidx[:],
                        channels=128, num_elems=2 * D, num_idxs=2 * NPAD)
                sbT[chk] = rot.tile([128, 8 * NSLAB * 128], F16,
                                    tag="sbT", name="sbT")
                nc.sync.dma_start(
                    out=sbT[chk][:, :tn * NSLAB * 128]
                        .rearrange("p (s q) -> p s q", q=128),
                    in_=askew[:, ta * D:tb * D],
                    transpose=True)

            def chunk_back(chk):
                """mm2 + out copies + out DMAs for one chunk."""
                ta, tb = CHB[chk], CHB[chk + 1]
                sb3 = sbT[chk][:].rearrange("p (s q) -> p s q", q=128)
                nout = 0
                ot = None
                for tp in range(ta, tb, 2):
                    if nout == 0:
                        t0 = tp
                        ot = outp.tile([128, 4 * C], F16, tag="ot", name="ot")
                    po = psp.tile([128, 2 * C], F32, tag="po", name="po", bufs=2)
                    for h in range(2):
                        t = tp + h
                        slabs = [s for s in range(NSLAB) if 0 <= t - 2 + s < NT]
                        for i, s in enumerate(slabs):
                            tq = t - 2 + s
                            nc.tensor.matmul(
                                po[:, h * C:(h + 1) * C],
                                sb3[:, (t - ta) * NSLAB + s, :],
                                yh[:, tq * C:(tq + 1) * C],
                                start=(i == 0), stop=(i == len(slabs) - 1))
                    nc.scalar.activation(ot[:, nout * C:(nout + 2) * C],
                                         po[:], AF.Copy)
                    nout += 2
                    if nout == 4 or tp + 2 == tb:
                        nc.sync.dma_start(
                            out=out_d[t0 * 128:(t0 + nout) * 128, :]
                                .rearrange("(h p) c -> p h c", p=128),
                            in_=ot[:, :nout * C].rearrange("p (h c) -> p h c", c=C))
                        nout = 0

            # ---- pipelined program ----
            # chunk_front(i) after conv group CHB[i+1]//2 - 1
            front_at = {CHB[i + 1] // 2 - 1: i for i in range(len(CHB) - 1)}
            back_at = {9: 0, 12: 1, 14: 2}
            for g in range(16):
                conv_group(g)
                if g >= 1 and 2 * g - 2 <= NT - 4:
                    mm1(2 * g - 2)
                if g in front_at:
                    chunk_front(front_at[g])
                if g in back_at:
                    chunk_back(back_at[g])
            mm1(30)
            chunk_back(3)
            chunk_back(4)

    return nc


_CACHE = {}
LAST_RESULT = None


def kernel(**inputs) -> np.ndarray:
    global LAST_RESULT
    x = np.asarray(inputs["x"]).astype(np.float16)
    B = x.shape[0]
    shared = {**_make_consts(),
              **_make_weights(inputs["w_off"], inputs["b_off"], inputs["w_mod"],
                              inputs["b_mod"], inputs["w_out"], inputs["b_out"])}

    if "nc" not in _CACHE:
        nc = bacc.Bacc("TRN2", target_bir_lowering=False, debug=False,
                       enable_asserts=False, num_devices=8)
        _build(nc)
        nc.finalize()
        _CACHE["nc"] = nc
    nc = _CACHE["nc"]

    in_maps = []
    for bi in range(B):
        m = dict(shared)
        m["x"] = np.ascontiguousarray(x[bi].reshape(C, HW))
        in_maps.append(m)

    res = bass_utils.run_bass_kernel_spmd(nc, in_maps, core_ids=list(range(B)))
    LAST_RESULT = res
    out = np.stack([r["out"] for r in res.results], 0).astype(np.float32)
    out = out.transpose(0, 2, 1).reshape(B, C, H, W)
    out = out + np.asarray(inputs["b_out"], np.float32)[None, :, None, None]
    return np.ascontiguousarray(out)


if __name__ == "__main__":
    import reference as R
    inp = {k: np.asarray(v) for k, v in R.setup_inputs().items()}
    got = kernel(**inp)
    print("kernel ran; output shape", got.shape)


# revision 19
# speedup vs baseline: 1.0413x; 1.0413x over previous
"""DCNv4 Trainium2 Bass kernel (v4, software-pipelined).

Data-parallel over batch: sample b runs on core b. Per-sample pipeline:
  1. conv-om via wide-stream implicit GEMM: per 4-row group and (ky, cb),
     one matmul with lhsT = [128c, 96] (3 kx-tap weight blocks at
     32-aligned columns) streaming 4 padded rows (264 cols) -> psum
     [96, 264].  x is DMA'd in 8 pieces so the conv starts early.
  2. merge+transpose+bias in ONE small matmul per 128-pixel tile:
     scalar casts the 3 shifted psum blocks to SBUF oms [97, 256] f16
     (row 96 = ones), then ptall[pix, 32t+o] = oms.T @ S97 where S97
     stacks [I32;I32;I32] + a bias row -> pixel-major offsets staying
     in PSUM (the DVE chain reads them there; no copy-out).  Offset
     channels are host-permuted (x 0..8, y 9..17, mod 18..26) so reads
     are stride-1.  mm1 (yT = (w_out @ x)^T) interleaves on the PE.
  3. bilinear math in f16 RELATIVE coords on DVE (f32-internal +2^23+16
     round trick; floor(x+off) = x + floor(off) since pixel coords are
     integers); exact bin-dedup via separable outer products
     A[p, 9sy, 9sx] = sum_k Ry_k (x) Cx_k, with k=7,8 on GPSIMD.
     Border validity is a CONSTANT mask applied to A in one multiply.
     Runs in 4 chunks of [10,10,8,4] tiles, each emitted right after
     its conv groups; the small last chunk shortens the serial tail.
  4. per 2-tile batch one collision-free GPSIMD local_scatter writes A
     into the skewed band Askew[p, d], d = p_local + 64*sy' + sx' + OFS;
     per chunk ONE XBAR DMA-transpose turns the slabs into sbT[q, s, p].
  5. per out-tile t, slab s: out2[p, c] += contraction(sbT slab,
     yT q-block) on the PE, placed a few groups after its chunk.
  6. scalar copies psum -> ot; 4-tile batched DMA to DRAM pixel-major
     [4096, 256]; host transposes and adds b_out.
"""

import sys

import numpy as np

for _p in ("/opt/trn_rl_repo",):
    if _p not in sys.path:
        sys.path.insert(0, _p)

import concourse.bass as bass
import concourse.mybir as mybir
from concourse import bacc
import concourse.tile as tile
from concourse import bass_utils

F32 = mybir.dt.float32
F16 = mybir.dt.float16
I16 = mybir.dt.int16

H = W = 64
HW = H * W
C = 256
NT = 32          # pixel tiles of 128 (2 image rows each)
NK = 9           # sample points
NB = 7           # bins per axis (shifts -3..3)
NBB = NB * NB
NPAD = 50        # per-tile A slots (49 bins + 1 pad)
OFS = 256        # skew offset; q = 128*t + d - OFS
D = 640          # skew width (5 slabs of 128)
NSLAB = 5
CHB = [0, 6, 14, 22, 28, 32]   # chunk tile boundaries (even)

# channel permutation: x-offsets, y-offsets, modulators contiguous
PERM = list(range(0, 18, 2)) + list(range(1, 18, 2)) + list(range(18, 27))


def _make_consts():
    iota2d = np.tile((np.arange(NB, dtype=np.float16) - 3.0)[:, None], (1, 2 * NK))
    pl = np.arange(128)
    sy, sx = np.meshgrid(np.arange(NB), np.arange(NB), indexing="ij")
    srel = (64 * (sy - 3) + (sx - 3) + OFS).reshape(-1)           # [49]
    scidx = np.full((128, 2, NPAD), -1, np.int16)
    for j in range(2):
        scidx[:, j, :NBB] = (pl[:, None] + srel[None, :] + j * D).astype(np.int16)
    # constant validity mask: pixel = 128*t + p -> y = 2t + (p>=64),
    # x = p % 64; corner at grid (sy, sx) has coords (y+sy-3, x+sx-3)
    t_ = np.arange(NT)
    yy = 2 * t_[None, :] + (pl[:, None] // 64)                    # [128, NT]
    xx = (pl % 64)[:, None] + np.zeros((1, NT), np.int64)         # [128, NT]
    cy = yy[:, :, None, None] + (sy - 3)[None, None]              # [128,NT,7,7]
    cx_ = xx[:, :, None, None] + (sx - 3)[None, None]
    mask = ((cy >= 0) & (cy < H) & (cx_ >= 0) & (cx_ < W)).astype(np.float16)
    return {
        "iota2d": np.ascontiguousarray(np.tile(iota2d.reshape(1, NB * 2 * NK), (128, 1))),
        "scidx": np.ascontiguousarray(scidx.reshape(128, 2 * NPAD)),
        "maskc": np.ascontiguousarray(mask.reshape(128, NT * NBB)),
    }


def _make_weights(w_off, b_off, w_mod, b_mod, w_out, b_out):
    wom = np.concatenate([np.asarray(w_off), np.asarray(w_mod)], 0)  # [27,256,3,3]
    wom = wom[PERM]
    bom = np.concatenate([np.asarray(b_off), np.asarray(b_mod)], 0)[PERM]
    # womt96[c, cb, ky, kx, o] = wom[o, cb*128+c, ky, kx]; each kx block
    # padded 27->32 columns so psum blocks land on 32-aligned partitions.
    w81 = np.transpose(wom.reshape(27, 2, 128, 3, 3), (2, 1, 3, 4, 0))
    w96 = np.zeros((128, 2, 3, 3, 32), np.float32)
    w96[:, :, :, :, :27] = w81
    # S97: [97, 32]; rows 32kx+o pick block kx col o; row 96 adds bias
    s97 = np.zeros((97, 32), np.float16)
    for kx in range(3):
        s97[32 * kx:32 * kx + 32, :] = np.eye(32, dtype=np.float16)
    s97[96, :27] = bom.astype(np.float16)
    woutt = np.asarray(w_out).reshape(C, C).T.copy()          # [cin, cout]
    return {
        "womt96": np.ascontiguousarray(w96.reshape(128, 2 * 3 * 96), np.float16),
        "s97": np.ascontiguousarray(s97),
        "woutt": np.ascontiguousarray(woutt, np.float16),
    }


def _build(nc: bass.Bass):
    AOp = mybir.AluOpType
    AF = mybir.ActivationFunctionType

    x_d = nc.dram_tensor("x", [C, HW], F16, kind="ExternalInput").ap()
    womt96_d = nc.dram_tensor("womt96", [128, 2 * 3 * 96], F16, kind="ExternalInput").ap()
    s97_d = nc.dram_tensor("s97", [97, 32], F16, kind="ExternalInput").ap()
    woutt_d = nc.dram_tensor("woutt", [C, C], F16, kind="ExternalInput").ap()
    iota_d = nc.dram_tensor("iota2d", [128, NB * 2 * NK], F16, kind="ExternalInput").ap()
    scidx_d = nc.dram_tensor("scidx", [128, 2 * NPAD], I16, kind="ExternalInput").ap()
    maskc_d = nc.dram_tensor("maskc", [128, NT * NBB], F16, kind="ExternalInput").ap()
    out_d = nc.dram_tensor("out", [HW, C], F16, kind="ExternalOutput").ap()

    with tile.TileContext(nc) as tc:
        with (
            tc.tile_pool(name="per", bufs=1) as per,
            tc.tile_pool(name="ps", bufs=1, space="PSUM") as psp,
            tc.tile_pool(name="rot", bufs=3) as rot,
            tc.tile_pool(name="outp", bufs=3) as outp,
        ):
            # persistent SBUF tensors
            xpad = [per.tile([128, 66 * 66], F16, tag=f"xpad{i}", name=f"xpad{i}") for i in range(2)]
            womt96 = per.tile([128, 2 * 3 * 96], F16, tag="womt96", name="womt96")
            s97 = per.tile([97, 32], F16, tag="s97", name="s97")
            woutt = per.tile([128, 2 * C], F16, tag="woutt", name="woutt")
            iota2 = per.tile([128, NB * 2 * NK], F16, tag="iota2", name="iota2")
            scidx = per.tile([128, 2 * NPAD], I16, tag="scidx", name="scidx")
            maskc = per.tile([128, NT * NBB], F16, tag="maskc", name="maskc")
            oms = per.tile([97, 2 * 256], F16, tag="oms", name="oms")
            yh = per.tile([128, NT * C], F16, tag="yh", name="yh")
            askew = per.tile([128, NT * D], F16, tag="askew", name="askew")
            xh = [per.tile([128, HW], F16, tag=f"xh{i}", name=f"xh{i}") for i in range(2)]
            ahd = per.tile([128, NT * NPAD], F16, tag="ahd", name="ahd")
            ahg = per.tile([128, NT * NBB], F16, tag="ahg", name="ahg")
            tta = per.tile([128, NT * NBB], F16, tag="tta", name="tta")
            ttg = per.tile([128, NT * NBB], F16, tag="ttg", name="ttg")
            # pixel-major conv outputs stay in PSUM ([p, 32t+o], f32)
            ptall = psp.tile([128, NT * 32], F32, tag="ptall", name="ptall")

            K2 = 2 * NK
            names = ("r2", "t02", "bxy", "fxy", "g2")
            b = {n: per.tile([128, NT * K2], F16, tag=f"b_{n}", name=f"b_{n}") for n in names}
            eq2 = per.tile([128, NT * NB * K2], F16, tag="eq2", name="eq2")
            t1c = per.tile([128, NT * (NB - 1) * K2], F16, tag="t1c", name="t1c")
            cxry = per.tile([128, NT * NB * K2], F16, tag="cxry", name="cxry")

            # conv weights first (small, needed immediately); x in 8
            # pieces per cb so xpad rows 0-8 and conv group 0 start after
            # ~1/8 of the transfer; xh doubles as mm1 lhsT.
            nc.sync.dma_start(out=womt96[:], in_=womt96_d)
            nc.sync.dma_start(out=s97[:], in_=s97_d)
            xsrc = x_d.rearrange("(cb p) q -> cb p q", p=128)
            for ch in range(8):
                q0, q1 = 512 * ch, 512 * (ch + 1)
                nc.sync.dma_start(out=xh[0][:, q0:q1], in_=xsrc[0][:, q0:q1])
                nc.scalar.dma_start(out=xh[1][:, q0:q1], in_=xsrc[1][:, q0:q1])
                if ch == 2:
                    nc.sync.dma_start(
                        out=woutt[:].rearrange("p (t o) -> p t o", o=C),
                        in_=woutt_d.rearrange("(t p) o -> p t o", p=128))
            for cb in range(2):
                x3 = xpad[cb][:].rearrange("p (y x) -> p y x", x=66)
                nc.vector.memset(x3[:, 0, :], 0.0)
                nc.vector.memset(x3[:, 65, :], 0.0)
                nc.vector.memset(x3[:, 1:65, 0], 0.0)
                nc.vector.memset(x3[:, 1:65, 65], 0.0)
            for ch in range(8):
                r0, r1 = 8 * ch, 8 * (ch + 1)
                for cb in range(2):
                    x3 = xpad[cb][:].rearrange("p (y x) -> p y x", x=66)
                    src = xh[cb][:].rearrange("p (y x) -> p y x", x=64)
                    nc.vector.tensor_copy(x3[:, 1 + r0:1 + r1, 1:65],
                                          src[:, r0:r1, :])

            # remaining constants (sync; needed only by the chunk pipeline)
            nc.sync.dma_start(out=iota2[:], in_=iota_d)
            nc.sync.dma_start(out=scidx[:], in_=scidx_d)
            nc.sync.dma_start(out=maskc[:], in_=maskc_d)

            nc.vector.memset(oms[96:97, :], 1.0)

            TT = nc.vector.tensor_tensor
            TS = nc.vector.tensor_scalar
            STT = nc.vector.scalar_tensor_tensor
            GTT = nc.gpsimd.tensor_tensor

            # pad slot (49) is read (and discarded) by the scatter
            nc.vector.memset(
                ahd[:].rearrange("p (t s) -> p t s", s=NPAD)[:, :, NBB], 0.0)

            omt3 = ptall[:].rearrange("p (t o) -> p t o", o=32)

            def conv_group(g):
                pom = psp.tile([96, 264], F32, tag="pom", name="pom", bufs=2)
                first = True
                for ky in range(3):
                    for cb in range(2):
                        lhsT = womt96[:, (cb * 3 + ky) * 96:(cb * 3 + ky + 1) * 96]
                        r0 = (4 * g + ky) * 66
                        rhs = xpad[cb][:, r0:r0 + 264]
                        nc.tensor.matmul(pom[:], lhsT, rhs, start=first,
                                         stop=(ky == 2 and cb == 1))
                        first = False
                base = (g % 2) * 256
                for kx in range(3):
                    pv = pom[32 * kx:32 * kx + 32, :] \
                        .rearrange("p (r c) -> p r c", c=66)[:, :, kx:kx + 64]
                    dst = oms[32 * kx:32 * kx + 32, base:base + 256] \
                        .rearrange("p (r c) -> p r c", c=64)
                    nc.scalar.activation(dst, pv, AF.Copy)
                for h2 in range(2):
                    t = 2 * g + h2
                    nc.tensor.matmul(ptall[:, t * 32:(t + 1) * 32],
                                     oms[:, base + h2 * 128:base + h2 * 128 + 128],
                                     s97[:], start=True, stop=True)

            def mm1(t):
                py = psp.tile([128, C], F32, tag="py", name="py", bufs=2)
                for cb in range(2):
                    lhsT = xh[cb][:, t * 128:(t + 1) * 128]
                    nc.tensor.matmul(py[:], lhsT, woutt[:, cb * C:(cb + 1) * C],
                                     start=(cb == 0), stop=(cb == 1))
                nc.scalar.activation(yh[:, t * C:(t + 1) * C], py[:], AF.Copy)

            sbT = {}

            def chunk_front(chk):
                """bilinear chain + eq + outer + scatter + transpose."""
                ta, tb = CHB[chk], CHB[chk + 1]
                tn = tb - ta
                tsl = slice(ta, tb)
                ksl = slice(ta * NK, tb * NK)
                bsl = slice(ta * NB * NK, tb * NB * NK)
                b1sl = slice(ta * (NB - 1) * NK, tb * (NB - 1) * NK)
                ssl = slice(ta * NBB, tb * NBB)

                om3c = omt3[:, tsl]
                oxy = om3c[:, :, 0:18]
                mmod = om3c[:, :, 18:27]
                K2 = 2 * NK
                ksl2 = slice(ta * K2, tb * K2)
                bsl2 = slice(ta * NB * K2, tb * NB * K2)
                b1sl2 = slice(ta * (NB - 1) * K2, tb * (NB - 1) * K2)
                fl = lambda ap_: ap_[:, ksl2]
                yv2 = lambda ap_: ap_[:, ksl2] \
                    .rearrange("p (t kk) -> p t kk", kk=K2)[:, :, NK:K2]

                # floor/frac for BOTH axes in one set of ops.  DVE computes
                # in fp32 internally: +2^23+16 forces rounding of s (in
                # (-4,4)) to an integer.
                RC = float(2 ** 23) + 16.0
                TS(fl(b["r2"][:]), oxy, RC, RC, AOp.add, AOp.subtract)
                STT(fl(b["t02"][:]), oxy, 0.0, fl(b["r2"][:]),
                    AOp.add, AOp.is_lt)
                TT(fl(b["bxy"][:]), fl(b["r2"][:]), fl(b["t02"][:]), AOp.subtract)
                TT(fl(b["fxy"][:]), oxy, fl(b["bxy"][:]), AOp.subtract)
                TS(fl(b["g2"][:]), fl(b["fxy"][:]), -1.0, 1.0, AOp.mult, AOp.add)
                # fold modulation into the y halves in place
                TT(yv2(b["g2"][:]), yv2(b["g2"][:]), mmod, AOp.mult)
                TT(yv2(b["fxy"][:]), yv2(b["fxy"][:]), mmod, AOp.mult)

                # eq + R/C for both axes at once: [128, t, bin, kk];
                # kk 0..8 -> Cx (x bins), kk 9..17 -> Ry (y bins, mod-folded)
                bkv = lambda ap_: ap_[:, bsl2] \
                    .rearrange("p (t b kk) -> p t b kk", b=NB, kk=K2)
                kv_b = lambda ap_: ap_[:, ksl2] \
                    .rearrange("p (t kk) -> p t kk", kk=K2) \
                    .unsqueeze(2).broadcast_to((128, tn, NB, K2))
                io_b = iota2[:].rearrange("q (b kk) -> q b kk", kk=K2) \
                    .unsqueeze(1).broadcast_to((128, tn, NB, K2))

                TT(bkv(eq2[:]), kv_b(b["bxy"][:]), io_b, AOp.is_equal)
                TT(bkv(cxry[:]), bkv(eq2[:]), kv_b(b["g2"][:]), AOp.mult)
                tv = t1c[:, b1sl2].rearrange("p (t b kk) -> p t b kk",
                                             b=NB - 1, kk=K2)
                TT(tv, bkv(eq2[:])[:, :, :NB - 1],
                   kv_b(b["fxy"][:])[:, :, :NB - 1], AOp.mult)
                TT(bkv(cxry[:])[:, :, 1:], bkv(cxry[:])[:, :, 1:], tv, AOp.add)

                # outer products: A[p, t, sy, sx] = sum_k ry_k (x) cx_k
                a_v = ahd[:].rearrange("p (t s) -> p t s", s=NPAD) \
                    [:, tsl, :NBB] \
                    .rearrange("p t (sy sx) -> p t sy sx", sy=NB, sx=NB)
                ag_v = ahg[:, ssl].rearrange("p (t sy sx) -> p t sy sx",
                                             sy=NB, sx=NB)
                ta_v = tta[:, ssl].rearrange("p (t sy sx) -> p t sy sx",
                                             sy=NB, sx=NB)
                tg_v = ttg[:, ssl].rearrange("p (t sy sx) -> p t sy sx",
                                             sy=NB, sx=NB)
                m_v = maskc[:, ssl].rearrange("p (t sy sx) -> p t sy sx",
                                              sy=NB, sx=NB)

                def ocx(k):
                    return bkv(cxry[:])[:, :, :, k].unsqueeze(2) \
                        .broadcast_to((128, tn, NB, NB))

                def ory(k):
                    return bkv(cxry[:])[:, :, :, NK + k].unsqueeze(3) \
                        .broadcast_to((128, tn, NB, NB))

                GTT(ag_v, ory(7), ocx(7), AOp.mult)
                GTT(tg_v, ory(8), ocx(8), AOp.mult)
                GTT(ag_v, ag_v, tg_v, AOp.add)
                GTT(tg_v, ory(6), ocx(6), AOp.mult)
                for k in range(6):
                    if k == 0:
                        TT(a_v, ory(0), ocx(0), AOp.mult)
                    else:
                        TT(ta_v, ory(k), ocx(k), AOp.mult)
                        TT(a_v, a_v, ta_v, AOp.add)
                # k=6 partial from GPSIMD, then k=7,8 partial + border mask
                TT(a_v, a_v, tg_v, AOp.add)
                TT(a_v, a_v, ag_v, AOp.add)
                TT(a_v, a_v, m_v, AOp.mult)

                # skewed scatters, then ONE batched XBAR transpose
                for bt in range(ta // 2, tb // 2):
                    nc.gpsimd.local_scatter(
                        askew[:, bt * 2 * D:(bt + 1) * 2 * D],
                        ahd[:, bt * 2 * NPAD:(bt + 1) * 2 * NPAD],
                        scidx[:],
                        channels=128, num_elems=2 * D, num_idxs=2 * NPAD)
                sbT[chk] = rot.tile([128, 8 * NSLAB * 128], F16,
                                    tag="sbT", name="sbT")
                nc.sync.dma_start(
                    out=sbT[chk][:, :tn * NSLAB * 128]
                        .rearrange("p (s q) -> p s q", q=128),
                    in_=askew[:, ta * D:tb * D],
                    transpose=True)

            def chunk_back(chk):
                """mm2 + out copies + out DMAs for one chunk."""
                ta, tb = CHB[chk], CHB[chk + 1]
                sb3 = sbT[chk][:].rearrange("p (s q) -> p s q", q=128)
                nout = 0
                ot = None
                for t in range(ta, tb):
                    if nout == 0:
                        t0 = t
                        ot = outp.tile([128, 4 * C], F16, tag="ot", name="ot")
                    po = psp.tile([128, C], F32, tag="po", name="po", bufs=2)
                    slabs = [s for s in range(NSLAB) if 0 <= t - 2 + s < NT]
                    for i, s in enumerate(slabs):
                        tq = t - 2 + s
                        nc.tensor.matmul(
                            po[:], sb3[:, (t - ta) * NSLAB + s, :],
                            yh[:, tq * C:(tq + 1) * C],
                            start=(i == 0), stop=(i == len(slabs) - 1))
                    nc.scalar.activation(ot[:, nout * C:(nout + 1) * C],
                                         po[:], AF.Copy)
                    nout += 1
                    if nout == 4 or t == tb - 1:
                        nc.sync.dma_start(
                            out=out_d[t0 * 128:(t0 + nout) * 128, :]
                                .rearrange("(h p) c -> p h c", p=128),
                            in_=ot[:, :nout * C].rearrange("p (h c) -> p h c", c=C))
                        nout = 0

            # ---- pipelined program ----
            # chunk_front(i) after conv group CHB[i+1]//2 - 1
            front_at = {CHB[i + 1] // 2 - 1: i for i in range(len(CHB) - 1)}
            back_at = {9: 0, 12: 1}
            for g in range(16):
                conv_group(g)
                if g >= 1 and 2 * g - 1 <= NT - 3:
                    mm1(2 * g - 2)
                    mm1(2 * g - 1)
                if g in front_at:
                    chunk_front(front_at[g])
                if g in back_at:
                    chunk_back(back_at[g])
            mm1(30)
            mm1(31)
            chunk_back(2)
            chunk_back(3)
            chunk_back(4)

    return nc


_CACHE = {}
LAST_RESULT = None


def kernel(**inputs) -> np.ndarray:
    global LAST_RESULT
    x = np.asarray(inputs["x"]).astype(np.float16)
    B = x.shape[0]
    shared = {**_make_consts(),
              **_make_weights(inputs["w_off"], inputs["b_off"], inputs["w_mod"],
                              inputs["b_mod"], inputs["w_out"], inputs["b_out"])}

    if "nc" not in _CACHE:
        nc = bacc.Bacc("TRN2", target_bir_lowering=False, debug=False,
                       enable_asserts=False, num_devices=8)
        _build(nc)
        nc.finalize()
        _CACHE["nc"] = nc
    nc = _CACHE["nc"]

    in_maps = []
    for bi in range(B):
        m = dict(shared)
        m["x"] = np.ascontiguousarray(x[bi].reshape(C, HW))
        in_maps.append(m)

    res = bass_utils.run_bass_kernel_spmd(nc, in_maps, core_ids=list(range(B)))
    LAST_RESULT = res
    out = np.stack([r["out"] for r in res.results], 0).astype(np.float32)
    out = out.transpose(0, 2, 1).reshape(B, C, H, W)
    out = out + np.asarray(inputs["b_out"], np.float32)[None, :, None, None]
    return np.ascontiguousarray(out)


if __name__ == "__main__":
    import reference as R
    inp = {k: np.asarray(v) for k, v in R.setup_inputs().items()}
    got = kernel(**inp)
    print("kernel ran; output shape", got.shape)
